# revision 19
# baseline (speedup 1.0000x reference)
"""Multi-head attention decode-block kernel for 8 Trainium2 NeuronCores.

Shapes (hardcoded from the problem spec):
  h:        [8, 16, 4096] f32
  Wq/Wk/Wv/Wo: [4096, 4096] f32 (nn.Linear convention: [out, in])
  K_cache/V_cache: [8, 32, 4096, 128] f32
  pos:      python int (2048)

Sharding: tensor-parallel over heads — 4 heads per core. Wq/Wk/Wv are
column-sharded, Wo row-sharded; each core computes a partial [128, 4096]
output and the host sums the 8 partials.
"""

import os
import sys

for _p in ("/opt/trn_rl_repo", "/root/.axon_site/_ro/trn_rl_repo"):
    if os.path.isdir(_p) and _p not in sys.path:
        sys.path.insert(0, _p)

from contextlib import ExitStack

import ml_dtypes
import numpy as np

import concourse.bacc as bacc
import concourse.bass as bass
import concourse.tile as tile
from concourse import mybir
from concourse.bass_utils import run_bass_kernel_spmd

BF16 = ml_dtypes.bfloat16

B, S, HIDDEN = 8, 16, 4096
NUM_HEADS, HEAD_DIM = 32, 128
N_CORES = 8
HPC = NUM_HEADS // N_CORES  # heads per core = 4
TOK = B * S  # 128 tokens
WCOL = HPC * HEAD_DIM  # 512 = per-core width of Wq/Wk/Wv (out) and Wo (in)
KC = HIDDEN // 128  # 32 contraction chunks for the projections
NG = 2  # head groups for the pipelined projection
GH = HPC // NG  # heads per group
GW = GH * HEAD_DIM  # projection output width per group

# Set by test harness to collect an NTFF profile; kernel() updates LAST_EXEC_NS.
TRACE = False
LAST_EXEC_NS = None

_PROGRAM_CACHE = {}


def _install_ntff_shim():
    """Register the antenv.axon_hooks NTFF hook if the image lacks it."""
    import types

    try:
        import antenv.axon_hooks  # noqa: F401

        return
    except ImportError:
        pass
    try:
        import antenv
        from trn_agent_boot.trn_boot import _ntff_profile_via_ctypes

        hook = _ntff_profile_via_ctypes("/opt/axon/libaxon_pjrt.so")
        mod = types.ModuleType("antenv.axon_hooks")
        mod._hook = hook
        mod.get_axon_ntff_profile_hook = lambda: hook
        mod.set_axon_ntff_profile_hook = lambda h: setattr(mod, "_hook", h)
        antenv.axon_hooks = mod
        sys.modules["antenv.axon_hooks"] = mod
    except Exception:
        pass


def _build_program(pos: int):
    """Build + compile the single-core Bass program (identical on all cores)."""
    n_full, rem = pos // 128, pos % 128
    # cache chunks: (t_offset, size); the fresh K/V block is handled separately
    chunks = [(c * 128, 128) for c in range(n_full)]
    if rem:
        chunks.append((n_full * 128, rem))
    n_ch = len(chunks)
    n_all = n_ch + 1  # + new block
    PW = n_all * 129  # per-pair width of the V tile
    f32 = mybir.dt.float32
    bf16 = mybir.dt.bfloat16
    inv_sqrt_hd = 1.0 / float(np.sqrt(HEAD_DIM))

    nc = bacc.Bacc("TRN2", target_bir_lowering=False, debug=False,
                   num_devices=N_CORES)

    ht = nc.dram_tensor("ht", [128, HIDDEN], bf16, kind="ExternalInput").ap()
    wqg = [nc.dram_tensor(f"wq{g}", [128, KC * GW], bf16,
                          kind="ExternalInput").ap() for g in range(NG)]
    wkg = [nc.dram_tensor(f"wk{g}", [128, KC * GW], bf16,
                          kind="ExternalInput").ap() for g in range(NG)]
    wvg = [nc.dram_tensor(f"wv{g}", [128, KC * GW], bf16,
                          kind="ExternalInput").ap() for g in range(NG)]
    wo = nc.dram_tensor("wo", [128, HPC * HIDDEN], bf16,
                        kind="ExternalInput").ap()
    kt = nc.dram_tensor("kt", [HPC, B, HEAD_DIM, pos], bf16,
                        kind="ExternalInput").ap() if pos else None
    va = nc.dram_tensor("va", [HPC, B, 128, n_ch, 129], bf16,
                        kind="ExternalInput").ap() if n_ch else None
    mask = nc.dram_tensor("mask", [S, S], bf16, kind="ExternalInput").ap()
    id16 = nc.dram_tensor("id16", [S, S], bf16, kind="ExternalInput").ap()
    id128 = nc.dram_tensor("id128", [128, 128], bf16, kind="ExternalInput").ap()
    out = nc.dram_tensor("out", [TOK, HIDDEN], f32, kind="ExternalOutput").ap()

    with tile.TileContext(nc) as tc, ExitStack() as ctx:
        const = ctx.enter_context(tc.tile_pool(name="const", bufs=1))
        dram = ctx.enter_context(tc.tile_pool(name="dram", bufs=1, space="DRAM"))

        ht_sb = const.tile([128, HIDDEN], bf16)
        nc.sync.dma_start(ht_sb[:], ht[:])
        mask_sb = const.tile([S, S], bf16)
        nc.sync.dma_start(mask_sb[:], mask[:])
        id16_sb = const.tile([S, S], bf16)
        nc.sync.dma_start(id16_sb[:], id16[:])
        id128_sb = const.tile([128, 128], bf16)
        nc.sync.dma_start(id128_sb[:], id128[:])

        # Per-head projection results, kept resident in SBUF.
        qt_sb = [const.tile([HEAD_DIM, TOK], bf16, tag=f"qt{h}", name=f"qt{h}")
                 for h in range(HPC)]
        ktn_sb = [const.tile([HEAD_DIM, TOK], bf16, tag=f"ktn{h}", name=f"ktn{h}")
                  for h in range(HPC)]
        aot_sb = [const.tile([HEAD_DIM, TOK], bf16, tag=f"aot{h}", name=f"aot{h}")
                  for h in range(HPC)]
        out_acc = const.tile([TOK, HIDDEN], f32)
        vnew_sb = const.tile([TOK, HPC * 129], bf16)
        ones_cols = vnew_sb[:].rearrange("p (c x) -> p c x", x=129)[:, :, 128:129]
        nc.vector.memset(ones_cols, 1.0)
        vnew_dram = dram.tile([TOK, HPC * 129], bf16)

        # All pools open up front. The projections borrow PSUM slots from the
        # attention pools (different pools -> different banks, so concurrent
        # accumulation groups never share a bank).
        ktp = ctx.enter_context(tc.tile_pool(name="ktp", bufs=4))
        vap = ctx.enter_context(tc.tile_pool(name="vap", bufs=4))
        expp = ctx.enter_context(tc.tile_pool(name="expp", bufs=3))
        smallp = ctx.enter_context(tc.tile_pool(name="smallp", bufs=4))
        toksb = ctx.enter_context(tc.tile_pool(name="toksb", bufs=2))
        wbig = ctx.enter_context(tc.tile_pool(name="wbig", bufs=4))
        wop = ctx.enter_context(tc.tile_pool(name="wop", bufs=2))
        spsum = ctx.enter_context(tc.tile_pool(name="spsum", bufs=2, space="PSUM"))
        opsum = ctx.enter_context(tc.tile_pool(name="opsum", bufs=2, space="PSUM"))
        tpsum = ctx.enter_context(tc.tile_pool(name="tpsum", bufs=2, space="PSUM"))
        wpsum = ctx.enter_context(tc.tile_pool(name="wpsum", bufs=2, space="PSUM"))

        # Weight streams ride the sync (hardware-DGE) ring: the software ring
        # is starved to ~20% while hardware queues are busy, so nothing
        # latency-critical goes there. Group 1's weights are fetched only
        # after head 0's cache stream is queued.
        wq_ts, wk_ts, wv_ts = [], [], []

        def fetch_weights(g):
            wq_t = wbig.tile([128, KC * GW], bf16, tag="wbig", name=f"wq_t{g}")
            nc.sync.dma_start(wq_t[:], wqg[g][:])
            wk_t = wbig.tile([128, KC * GW], bf16, tag="wbig", name=f"wk_t{g}")
            nc.sync.dma_start(wk_t[:], wkg[g][:])
            wv_t = wbig.tile([128, KC * GW], bf16, tag="wbig", name=f"wv_t{g}")
            nc.sync.dma_start(wv_t[:], wvg[g][:])
            wq_ts.append(wq_t)
            wk_ts.append(wk_t)
            wv_ts.append(wv_t)

        fetch_weights(0)

        def proj_group(g):
            """Q/K/V projection for head group g, borrowing attention PSUM."""
            psq = spsum.tile([TOK, GW], f32, tag="sc", name=f"psq{g}")
            psk = opsum.tile([TOK, GW], f32, tag="ou", name=f"psk{g}")
            psv = wpsum.tile([TOK, GW], f32, tag="wp", name=f"psv{g}")
            # One loop per tensor so Q's matmuls begin as soon as wq lands
            # (the interleaved form waits for all three weight tensors).
            for ps, wt in ((psq, wq_ts[g]), (psk, wk_ts[g]), (psv, wv_ts[g])):
                for c in range(KC):
                    nc.tensor.matmul(ps[:], ht_sb[:, c * 128:(c + 1) * 128],
                                     wt[:, c * GW:(c + 1) * GW],
                                     start=(c == 0), stop=(c == KC - 1))

            for ps, dests in ((psq, qt_sb), (psk, ktn_sb)):
                tok_t = toksb.tile([TOK, GW], bf16, tag="tok")
                nc.scalar.activation(tok_t[:], ps[:],
                                     mybir.ActivationFunctionType.Copy)
                for j in range(GH):
                    h = g * GH + j
                    tpp = tpsum.tile([HEAD_DIM, TOK], bf16, tag="tp",
                                     name=f"tpp{g}{j}")
                    nc.tensor.transpose(
                        tpp[:], tok_t[:, j * HEAD_DIM:(j + 1) * HEAD_DIM],
                        id128_sb[:])
                    nc.scalar.activation(dests[h][:], tpp[:],
                                         mybir.ActivationFunctionType.Copy)

            # V: bounce through DRAM so each (b, h) slice (with its ones
            # column) can later be DMA'd to partitions 0..15.
            for j in range(GH):
                h = g * GH + j
                nc.scalar.activation(
                    vnew_sb[:, h * 129:h * 129 + HEAD_DIM],
                    psv[:, j * HEAD_DIM:(j + 1) * HEAD_DIM],
                    mybir.ActivationFunctionType.Copy)
            gsl = slice(g * GH * 129, (g + 1) * GH * 129)
            nc.scalar.dma_start(vnew_dram[:, gsl], vnew_sb[:, gsl])

        def wo_fetch(h2):
            t = wop.tile([128, HIDDEN], bf16, tag="wo", name=f"wo{h2}")
            nc.gpsimd.dma_start(t[:], wo[:, h2 * HIDDEN:(h2 + 1) * HIDDEN])
            return t

        proj_group(0)
        wo_sb = wo_fetch(0)

        for h in range(HPC):
            for bb in range(0, B, 2):
                # two pairs per cache DMA
                if pos:
                    kt2 = ktp.tile([128, 2 * pos], bf16, tag="kt")
                    nc.sync.dma_start(
                        kt2[:].rearrange("p (b t) -> p b t", b=2),
                        kt[h, bb:bb + 2].rearrange("b p t -> p b t"))
                va2 = vap.tile([128, 2 * PW], bf16, tag="va")
                if n_ch:
                    dstv = va2[:].rearrange("p (b z) -> p b z", z=PW)
                    nc.gpsimd.dma_start(
                        dstv[:, :, :n_ch * 129],
                        va[h, bb:bb + 2].rearrange("b p c x -> p b c x")
                        .rearrange("p b c x -> p b (c x)"))
                for b in (bb, bb + 1):
                    ts = b * S
                    po = (b - bb) * PW  # offset of this pair in va2
                    ko = (b - bb) * pos  # offset of this pair in kt2
                    nc.sync.dma_start(
                        va2[:S, po + n_ch * 129:po + n_all * 129],
                        vnew_dram[ts:ts + S, h * 129:(h + 1) * 129])

                    sc = spsum.tile([128, n_all * S], f32, tag="sc")
                    for ci, (t0, tsz) in enumerate(chunks):
                        nc.tensor.matmul(sc[:tsz, ci * S:(ci + 1) * S],
                                         kt2[:, ko + t0:ko + t0 + tsz],
                                         qt_sb[h][:, ts:ts + S],
                                         start=True, stop=True)
                    nc.tensor.matmul(sc[:S, n_ch * S:n_all * S],
                                     ktn_sb[h][:, ts:ts + S],
                                     qt_sb[h][:, ts:ts + S],
                                     start=True, stop=True)

                    # exp((q.k)/sqrt(hd)); scores ~N(0,1) so no max-shift.
                    ex = expp.tile([128, n_all * S], bf16, tag="ex")
                    if n_full:
                        nc.scalar.activation(ex[:, :n_full * S],
                                             sc[:, :n_full * S],
                                             mybir.ActivationFunctionType.Exp,
                                             scale=inv_sqrt_hd)
                    if rem:
                        nc.scalar.activation(ex[:rem, n_full * S:n_ch * S],
                                             sc[:rem, n_full * S:n_ch * S],
                                             mybir.ActivationFunctionType.Exp,
                                             scale=inv_sqrt_hd)
                    nc.scalar.activation(ex[:S, n_ch * S:n_all * S],
                                         sc[:S, n_ch * S:n_all * S],
                                         mybir.ActivationFunctionType.Exp,
                                         scale=inv_sqrt_hd)
                    nc.vector.tensor_mul(ex[:S, n_ch * S:n_all * S],
                                         ex[:S, n_ch * S:n_all * S], mask_sb[:])

                    # out[s, :128] = sum_t exp * V ; col 128 = sum_t exp
                    ou = opsum.tile([S, 129], f32, tag="ou")
                    for ci, (t0, tsz) in enumerate(chunks):
                        nc.tensor.matmul(
                            ou[:], ex[:tsz, ci * S:(ci + 1) * S],
                            va2[:tsz, po + ci * 129:po + ci * 129 + 129],
                            start=(ci == 0), stop=False)
                    nc.tensor.matmul(ou[:], ex[:S, n_ch * S:n_all * S],
                                     va2[:S, po + n_ch * 129:po + n_all * 129],
                                     start=(n_ch == 0), stop=True)

                    rd = smallp.tile([S, 1], f32, tag="rd")
                    nc.vector.reciprocal(rd[:], ou[:, 128:129])
                    aon = smallp.tile([S, HEAD_DIM], bf16, tag="aon")
                    nc.vector.tensor_scalar_mul(aon[:], ou[:, :HEAD_DIM], rd[:])

                    tp = tpsum.tile([HEAD_DIM, S], bf16, tag="tp")
                    nc.tensor.transpose(tp[:], aon[:], id16_sb[:])
                    nc.scalar.activation(aot_sb[h][:, ts:ts + S], tp[:],
                                         mybir.ActivationFunctionType.Copy)

            if h == 0:
                fetch_weights(1)
                proj_group(1)

            # Output projection for this head (row-sharded Wo), accumulated
            # into out_acc on the vector engine.
            for ncv in range(HIDDEN // 512):
                osl = slice(ncv * 512, (ncv + 1) * 512)
                wp = wpsum.tile([TOK, 512], f32, tag="wp")
                nc.tensor.matmul(wp[:], aot_sb[h][:],
                                 wo_sb[:, osl], start=True, stop=True)
                if h == 0:
                    nc.vector.tensor_copy(out_acc[:, osl], wp[:])
                else:
                    nc.vector.tensor_add(out_acc[:, osl], out_acc[:, osl], wp[:])
            if h + 1 < HPC:
                wo_sb = wo_fetch(h + 1)

        for ncv in range(HIDDEN // 512):
            eng = nc.sync if ncv % 2 == 0 else nc.gpsimd
            eng.dma_start(out[:, ncv * 512:(ncv + 1) * 512],
                          out_acc[:, ncv * 512:(ncv + 1) * 512])

    nc.compile()
    return nc


def kernel(h, Wq, Wk, Wv, Wo, K_cache, V_cache, pos):
    global LAST_EXEC_NS
    pos = int(pos)

    h = np.asarray(h, dtype=np.float32)
    Wq = np.asarray(Wq, dtype=np.float32)
    Wk = np.asarray(Wk, dtype=np.float32)
    Wv = np.asarray(Wv, dtype=np.float32)
    Wo = np.asarray(Wo, dtype=np.float32)
    K_cache = np.asarray(K_cache, dtype=np.float32)
    V_cache = np.asarray(V_cache, dtype=np.float32)

    n_full, rem = pos // 128, pos % 128
    n_ch = n_full + (1 if rem else 0)

    hf = h.reshape(TOK, HIDDEN)
    # ht_sb[p, c*128 + t] = hf[t, c*128 + p]
    ht_np = np.ascontiguousarray(
        hf.T.reshape(KC, 128, TOK).transpose(1, 0, 2).reshape(128, HIDDEN)
    ).astype(BF16)
    mask_np = (np.arange(S)[:, None] <= np.arange(S)[None, :]).astype(BF16)
    id16_np = np.eye(S, dtype=np.float32).astype(BF16)
    id128_np = np.eye(128, dtype=np.float32).astype(BF16)

    def wlayout(wT):  # [4096, n] -> [128, 32*n]; w_sb[p, c*n + j] = wT[c*128+p, j]
        n = wT.shape[1]
        return np.ascontiguousarray(
            wT.reshape(KC, 128, n).transpose(1, 0, 2).reshape(128, KC * n))

    in_maps = []
    for c in range(N_CORES):
        hs = c * HPC  # first head of this core
        r0, r1 = hs * HEAD_DIM, (hs + HPC) * HEAD_DIM
        woT = Wo[:, r0:r1].T  # [512, 4096]
        m = {
            "ht": ht_np,
            "wo": np.ascontiguousarray(
                woT.reshape(HPC, 128, HIDDEN).transpose(1, 0, 2)
                .reshape(128, HPC * HIDDEN)).astype(BF16),
            "mask": mask_np,
            "id16": id16_np,
            "id128": id128_np,
        }
        for g in range(NG):
            g0 = r0 + g * GW
            m[f"wq{g}"] = wlayout(Wq[g0:g0 + GW, :].T).astype(BF16)
            m[f"wk{g}"] = wlayout(Wk[g0:g0 + GW, :].T).astype(BF16)
            m[f"wv{g}"] = wlayout(Wv[g0:g0 + GW, :].T).astype(BF16)
        if pos:
            m["kt"] = np.ascontiguousarray(
                K_cache[:, hs:hs + HPC, :pos, :].transpose(1, 0, 3, 2)
            ).astype(BF16)
        if n_ch:
            vsl = V_cache[:, hs:hs + HPC, :n_ch * 128, :]
            if rem:
                vsl = np.concatenate(
                    [V_cache[:, hs:hs + HPC, :pos, :],
                     np.zeros((B, HPC, n_ch * 128 - pos, HEAD_DIM), np.float32)],
                    axis=2)
            vperm = (vsl.reshape(B, HPC, n_ch, 128, HEAD_DIM)
                     .transpose(1, 0, 3, 2, 4))  # [h, b, p, c, j]
            vaug = np.ones((HPC, B, 128, n_ch, 129), np.float32)
            vaug[..., :HEAD_DIM] = vperm
            m["va"] = vaug.astype(BF16)
        in_maps.append(m)

    if pos not in _PROGRAM_CACHE:
        _PROGRAM_CACHE[pos] = _build_program(pos)
    nc = _PROGRAM_CACHE[pos]

    if TRACE:
        _install_ntff_shim()
    res = run_bass_kernel_spmd(nc, in_maps, list(range(N_CORES)), trace=TRACE)
    LAST_EXEC_NS = res.exec_time_ns

    acc = np.zeros((TOK, HIDDEN), np.float32)
    for r in res.results:
        acc += np.asarray(r["out"], np.float32)
    return acc.reshape(B, S, HIDDEN)


# revision 20
# speedup vs baseline: 1.0861x; 1.0861x over previous
"""Multi-head attention decode-block kernel for 8 Trainium2 NeuronCores.

Shapes (hardcoded from the problem spec):
  h:        [8, 16, 4096] f32
  Wq/Wk/Wv/Wo: [4096, 4096] f32 (nn.Linear convention: [out, in])
  K_cache/V_cache: [8, 32, 4096, 128] f32
  pos:      python int (2048)

Sharding: tensor-parallel over heads — 4 heads per core. Wq/Wk/Wv are
column-sharded, Wo row-sharded; each core computes a partial [128, 4096]
output and the host sums the 8 partials.
"""

import os
import sys

for _p in ("/opt/trn_rl_repo", "/root/.axon_site/_ro/trn_rl_repo"):
    if os.path.isdir(_p) and _p not in sys.path:
        sys.path.insert(0, _p)

from contextlib import ExitStack

import ml_dtypes
import numpy as np

import concourse.bacc as bacc
import concourse.bass as bass
import concourse.tile as tile
from concourse import mybir
from concourse.bass_utils import run_bass_kernel_spmd

BF16 = ml_dtypes.bfloat16

B, S, HIDDEN = 8, 16, 4096
NUM_HEADS, HEAD_DIM = 32, 128
N_CORES = 8
HPC = NUM_HEADS // N_CORES  # heads per core = 4
TOK = B * S  # 128 tokens
WCOL = HPC * HEAD_DIM  # 512 = per-core width of Wq/Wk/Wv (out) and Wo (in)
KC = HIDDEN // 128  # 32 contraction chunks for the projections
NG = 2  # head groups for the pipelined projection
GH = HPC // NG  # heads per group
GW = GH * HEAD_DIM  # projection output width per group

# Set by test harness to collect an NTFF profile; kernel() updates LAST_EXEC_NS.
TRACE = False
LAST_EXEC_NS = None

_PROGRAM_CACHE = {}


def _install_ntff_shim():
    """Register the antenv.axon_hooks NTFF hook if the image lacks it."""
    import types

    try:
        import antenv.axon_hooks  # noqa: F401

        return
    except ImportError:
        pass
    try:
        import antenv
        from trn_agent_boot.trn_boot import _ntff_profile_via_ctypes

        hook = _ntff_profile_via_ctypes("/opt/axon/libaxon_pjrt.so")
        mod = types.ModuleType("antenv.axon_hooks")
        mod._hook = hook
        mod.get_axon_ntff_profile_hook = lambda: hook
        mod.set_axon_ntff_profile_hook = lambda h: setattr(mod, "_hook", h)
        antenv.axon_hooks = mod
        sys.modules["antenv.axon_hooks"] = mod
    except Exception:
        pass


def _build_program(pos: int):
    """Build + compile the single-core Bass program (identical on all cores)."""
    n_full, rem = pos // 128, pos % 128
    # cache chunks: (t_offset, size); the fresh K/V block is handled separately
    chunks = [(c * 128, 128) for c in range(n_full)]
    if rem:
        chunks.append((n_full * 128, rem))
    n_ch = len(chunks)
    n_all = n_ch + 1  # + new block
    PW = n_all * 129  # per-pair width of the V tile
    f32 = mybir.dt.float32
    bf16 = mybir.dt.bfloat16
    inv_sqrt_hd = 1.0 / float(np.sqrt(HEAD_DIM))

    nc = bacc.Bacc("TRN2", target_bir_lowering=False, debug=False,
                   num_devices=N_CORES)

    ht = nc.dram_tensor("ht", [128, HIDDEN], bf16, kind="ExternalInput").ap()
    wqg = [nc.dram_tensor(f"wq{g}", [128, KC * GW], bf16,
                          kind="ExternalInput").ap() for g in range(NG)]
    wkg = [nc.dram_tensor(f"wk{g}", [128, KC * GW], bf16,
                          kind="ExternalInput").ap() for g in range(NG)]
    wvg = [nc.dram_tensor(f"wv{g}", [128, KC * GW], bf16,
                          kind="ExternalInput").ap() for g in range(NG)]
    wo = nc.dram_tensor("wo", [128, HPC * HIDDEN], bf16,
                        kind="ExternalInput").ap()
    kt = nc.dram_tensor("kt", [HPC, B, HEAD_DIM, pos], bf16,
                        kind="ExternalInput").ap() if pos else None
    va = nc.dram_tensor("va", [HPC, B, 128, n_ch, 129], bf16,
                        kind="ExternalInput").ap() if n_ch else None
    mask = nc.dram_tensor("mask", [S, S], bf16, kind="ExternalInput").ap()
    id16 = nc.dram_tensor("id16", [S, S], bf16, kind="ExternalInput").ap()
    id128 = nc.dram_tensor("id128", [128, 128], bf16, kind="ExternalInput").ap()
    out = nc.dram_tensor("out", [TOK, HIDDEN], f32, kind="ExternalOutput").ap()

    with tile.TileContext(nc) as tc, ExitStack() as ctx:
        const = ctx.enter_context(tc.tile_pool(name="const", bufs=1))
        dram = ctx.enter_context(tc.tile_pool(name="dram", bufs=1, space="DRAM"))

        ht_sb = const.tile([128, HIDDEN], bf16)
        nc.sync.dma_start(ht_sb[:], ht[:])
        mask_sb = const.tile([S, S], bf16)
        nc.sync.dma_start(mask_sb[:], mask[:])
        id16_sb = const.tile([S, S], bf16)
        nc.sync.dma_start(id16_sb[:], id16[:])
        id128_sb = const.tile([128, 128], bf16)
        nc.sync.dma_start(id128_sb[:], id128[:])

        # Per-head projection results, kept resident in SBUF.
        qt_sb = [const.tile([HEAD_DIM, TOK], bf16, tag=f"qt{h}", name=f"qt{h}")
                 for h in range(HPC)]
        ktn_sb = [const.tile([HEAD_DIM, TOK], bf16, tag=f"ktn{h}", name=f"ktn{h}")
                  for h in range(HPC)]
        aot_sb = [const.tile([HEAD_DIM, TOK], bf16, tag=f"aot{h}", name=f"aot{h}")
                  for h in range(HPC)]
        out_acc = const.tile([TOK, HIDDEN], f32)
        vnew_sb = const.tile([TOK, HPC * 129], bf16)
        ones_cols = vnew_sb[:].rearrange("p (c x) -> p c x", x=129)[:, :, 128:129]
        nc.vector.memset(ones_cols, 1.0)
        vnew_dram = dram.tile([TOK, HPC * 129], bf16)

        # All pools open up front. The projections borrow PSUM slots from the
        # attention pools (different pools -> different banks, so concurrent
        # accumulation groups never share a bank).
        ktp = ctx.enter_context(tc.tile_pool(name="ktp", bufs=4))
        vap = ctx.enter_context(tc.tile_pool(name="vap", bufs=4))
        expp = ctx.enter_context(tc.tile_pool(name="expp", bufs=3))
        smallp = ctx.enter_context(tc.tile_pool(name="smallp", bufs=4))
        toksb = ctx.enter_context(tc.tile_pool(name="toksb", bufs=2))
        wbig = ctx.enter_context(tc.tile_pool(name="wbig", bufs=4))
        wop = ctx.enter_context(tc.tile_pool(name="wop", bufs=2))
        spsum = ctx.enter_context(tc.tile_pool(name="spsum", bufs=2, space="PSUM"))
        opsum = ctx.enter_context(tc.tile_pool(name="opsum", bufs=2, space="PSUM"))
        tpsum = ctx.enter_context(tc.tile_pool(name="tpsum", bufs=2, space="PSUM"))
        wpsum = ctx.enter_context(tc.tile_pool(name="wpsum", bufs=2, space="PSUM"))

        # Weight streams ride the sync (hardware-DGE) ring: the software ring
        # is starved to ~20% while hardware queues are busy, so nothing
        # latency-critical goes there. Group 1's weights are fetched only
        # after head 0's cache stream is queued.
        wq_ts, wk_ts, wv_ts = [], [], []

        def fetch_weights(g):
            wq_t = wbig.tile([128, KC * GW], bf16, tag="wbig", name=f"wq_t{g}")
            nc.sync.dma_start(wq_t[:], wqg[g][:])
            wk_t = wbig.tile([128, KC * GW], bf16, tag="wbig", name=f"wk_t{g}")
            nc.sync.dma_start(wk_t[:], wkg[g][:])
            wv_t = wbig.tile([128, KC * GW], bf16, tag="wbig", name=f"wv_t{g}")
            nc.sync.dma_start(wv_t[:], wvg[g][:])
            wq_ts.append(wq_t)
            wk_ts.append(wk_t)
            wv_ts.append(wv_t)

        fetch_weights(0)

        def proj_group(g):
            """Q/K/V projection for head group g, borrowing attention PSUM."""
            psq = spsum.tile([TOK, GW], f32, tag="sc", name=f"psq{g}")
            psk = opsum.tile([TOK, GW], f32, tag="ou", name=f"psk{g}")
            psv = wpsum.tile([TOK, GW], f32, tag="wp", name=f"psv{g}")
            for c in range(KC):
                hc = ht_sb[:, c * 128:(c + 1) * 128]
                wsl = slice(c * GW, (c + 1) * GW)
                nc.tensor.matmul(psq[:], hc, wq_ts[g][:, wsl],
                                 start=(c == 0), stop=(c == KC - 1))
                nc.tensor.matmul(psk[:], hc, wk_ts[g][:, wsl],
                                 start=(c == 0), stop=(c == KC - 1))
                nc.tensor.matmul(psv[:], hc, wv_ts[g][:, wsl],
                                 start=(c == 0), stop=(c == KC - 1))

            for ps, dests in ((psq, qt_sb), (psk, ktn_sb)):
                tok_t = toksb.tile([TOK, GW], bf16, tag="tok")
                nc.scalar.activation(tok_t[:], ps[:],
                                     mybir.ActivationFunctionType.Copy)
                for j in range(GH):
                    h = g * GH + j
                    tpp = tpsum.tile([HEAD_DIM, TOK], bf16, tag="tp",
                                     name=f"tpp{g}{j}")
                    nc.tensor.transpose(
                        tpp[:], tok_t[:, j * HEAD_DIM:(j + 1) * HEAD_DIM],
                        id128_sb[:])
                    nc.scalar.activation(dests[h][:], tpp[:],
                                         mybir.ActivationFunctionType.Copy)

            # V: bounce through DRAM so each (b, h) slice (with its ones
            # column) can later be DMA'd to partitions 0..15.
            for j in range(GH):
                h = g * GH + j
                nc.scalar.activation(
                    vnew_sb[:, h * 129:h * 129 + HEAD_DIM],
                    psv[:, j * HEAD_DIM:(j + 1) * HEAD_DIM],
                    mybir.ActivationFunctionType.Copy)
            gsl = slice(g * GH * 129, (g + 1) * GH * 129)
            nc.scalar.dma_start(vnew_dram[:, gsl], vnew_sb[:, gsl])

        def wo_fetch(h2):
            t = wop.tile([128, HIDDEN], bf16, tag="wo", name=f"wo{h2}")
            nc.gpsimd.dma_start(t[:], wo[:, h2 * HIDDEN:(h2 + 1) * HIDDEN])
            return t

        proj_group(0)
        wo_sb = wo_fetch(0)

        for h in range(HPC):
            for bb in range(0, B, 2):
                # two pairs per cache DMA
                if pos:
                    kt2 = ktp.tile([128, 2 * pos], bf16, tag="kt")
                    nc.sync.dma_start(
                        kt2[:].rearrange("p (b t) -> p b t", b=2),
                        kt[h, bb:bb + 2].rearrange("b p t -> p b t"))
                va2 = vap.tile([128, 2 * PW], bf16, tag="va")
                if n_ch:
                    dstv = va2[:].rearrange("p (b z) -> p b z", z=PW)
                    nc.gpsimd.dma_start(
                        dstv[:, :, :n_ch * 129],
                        va[h, bb:bb + 2].rearrange("b p c x -> p b c x")
                        .rearrange("p b c x -> p b (c x)"))
                for b in (bb, bb + 1):
                    ts = b * S
                    po = (b - bb) * PW  # offset of this pair in va2
                    ko = (b - bb) * pos  # offset of this pair in kt2
                    nc.sync.dma_start(
                        va2[:S, po + n_ch * 129:po + n_all * 129],
                        vnew_dram[ts:ts + S, h * 129:(h + 1) * 129])

                    sc = spsum.tile([128, n_all * S], f32, tag="sc")
                    for ci, (t0, tsz) in enumerate(chunks):
                        nc.tensor.matmul(sc[:tsz, ci * S:(ci + 1) * S],
                                         kt2[:, ko + t0:ko + t0 + tsz],
                                         qt_sb[h][:, ts:ts + S],
                                         start=True, stop=True)
                    nc.tensor.matmul(sc[:S, n_ch * S:n_all * S],
                                     ktn_sb[h][:, ts:ts + S],
                                     qt_sb[h][:, ts:ts + S],
                                     start=True, stop=True)

                    # exp((q.k)/sqrt(hd)); scores ~N(0,1) so no max-shift.
                    ex = expp.tile([128, n_all * S], bf16, tag="ex")
                    if n_full:
                        nc.scalar.activation(ex[:, :n_full * S],
                                             sc[:, :n_full * S],
                                             mybir.ActivationFunctionType.Exp,
                                             scale=inv_sqrt_hd)
                    if rem:
                        nc.scalar.activation(ex[:rem, n_full * S:n_ch * S],
                                             sc[:rem, n_full * S:n_ch * S],
                                             mybir.ActivationFunctionType.Exp,
                                             scale=inv_sqrt_hd)
                    nc.scalar.activation(ex[:S, n_ch * S:n_all * S],
                                         sc[:S, n_ch * S:n_all * S],
                                         mybir.ActivationFunctionType.Exp,
                                         scale=inv_sqrt_hd)
                    nc.vector.tensor_mul(ex[:S, n_ch * S:n_all * S],
                                         ex[:S, n_ch * S:n_all * S], mask_sb[:])

                    # out[s, :128] = sum_t exp * V ; col 128 = sum_t exp
                    ou = opsum.tile([S, 129], f32, tag="ou")
                    for ci, (t0, tsz) in enumerate(chunks):
                        nc.tensor.matmul(
                            ou[:], ex[:tsz, ci * S:(ci + 1) * S],
                            va2[:tsz, po + ci * 129:po + ci * 129 + 129],
                            start=(ci == 0), stop=False)
                    nc.tensor.matmul(ou[:], ex[:S, n_ch * S:n_all * S],
                                     va2[:S, po + n_ch * 129:po + n_all * 129],
                                     start=(n_ch == 0), stop=True)

                    rd = smallp.tile([S, 1], f32, tag="rd")
                    nc.vector.reciprocal(rd[:], ou[:, 128:129])
                    aon = smallp.tile([S, HEAD_DIM], bf16, tag="aon")
                    nc.vector.tensor_scalar_mul(aon[:], ou[:, :HEAD_DIM], rd[:])

                    tp = tpsum.tile([HEAD_DIM, S], bf16, tag="tp")
                    nc.tensor.transpose(tp[:], aon[:], id16_sb[:])
                    nc.scalar.activation(aot_sb[h][:, ts:ts + S], tp[:],
                                         mybir.ActivationFunctionType.Copy)

            if h == 0:
                fetch_weights(1)
                proj_group(1)

            # Output projection for this head (row-sharded Wo), accumulated
            # into out_acc on the vector engine.
            for ncv in range(HIDDEN // 512):
                osl = slice(ncv * 512, (ncv + 1) * 512)
                wp = wpsum.tile([TOK, 512], f32, tag="wp")
                nc.tensor.matmul(wp[:], aot_sb[h][:],
                                 wo_sb[:, osl], start=True, stop=True)
                if h == 0:
                    nc.vector.tensor_copy(out_acc[:, osl], wp[:])
                else:
                    nc.vector.tensor_add(out_acc[:, osl], out_acc[:, osl], wp[:])
            if h + 1 < HPC:
                wo_sb = wo_fetch(h + 1)

        for ncv in range(HIDDEN // 512):
            eng = nc.sync if ncv % 2 == 0 else nc.gpsimd
            eng.dma_start(out[:, ncv * 512:(ncv + 1) * 512],
                          out_acc[:, ncv * 512:(ncv + 1) * 512])

    nc.compile()
    return nc


def kernel(h, Wq, Wk, Wv, Wo, K_cache, V_cache, pos):
    global LAST_EXEC_NS
    pos = int(pos)

    h = np.asarray(h, dtype=np.float32)
    Wq = np.asarray(Wq, dtype=np.float32)
    Wk = np.asarray(Wk, dtype=np.float32)
    Wv = np.asarray(Wv, dtype=np.float32)
    Wo = np.asarray(Wo, dtype=np.float32)
    K_cache = np.asarray(K_cache, dtype=np.float32)
    V_cache = np.asarray(V_cache, dtype=np.float32)

    n_full, rem = pos // 128, pos % 128
    n_ch = n_full + (1 if rem else 0)

    hf = h.reshape(TOK, HIDDEN)
    # ht_sb[p, c*128 + t] = hf[t, c*128 + p]
    ht_np = np.ascontiguousarray(
        hf.T.reshape(KC, 128, TOK).transpose(1, 0, 2).reshape(128, HIDDEN)
    ).astype(BF16)
    mask_np = (np.arange(S)[:, None] <= np.arange(S)[None, :]).astype(BF16)
    id16_np = np.eye(S, dtype=np.float32).astype(BF16)
    id128_np = np.eye(128, dtype=np.float32).astype(BF16)

    def wlayout(wT):  # [4096, n] -> [128, 32*n]; w_sb[p, c*n + j] = wT[c*128+p, j]
        n = wT.shape[1]
        return np.ascontiguousarray(
            wT.reshape(KC, 128, n).transpose(1, 0, 2).reshape(128, KC * n))

    in_maps = []
    for c in range(N_CORES):
        hs = c * HPC  # first head of this core
        r0, r1 = hs * HEAD_DIM, (hs + HPC) * HEAD_DIM
        woT = Wo[:, r0:r1].T  # [512, 4096]
        m = {
            "ht": ht_np,
            "wo": np.ascontiguousarray(
                woT.reshape(HPC, 128, HIDDEN).transpose(1, 0, 2)
                .reshape(128, HPC * HIDDEN)).astype(BF16),
            "mask": mask_np,
            "id16": id16_np,
            "id128": id128_np,
        }
        for g in range(NG):
            g0 = r0 + g * GW
            m[f"wq{g}"] = wlayout(Wq[g0:g0 + GW, :].T).astype(BF16)
            m[f"wk{g}"] = wlayout(Wk[g0:g0 + GW, :].T).astype(BF16)
            m[f"wv{g}"] = wlayout(Wv[g0:g0 + GW, :].T).astype(BF16)
        if pos:
            m["kt"] = np.ascontiguousarray(
                K_cache[:, hs:hs + HPC, :pos, :].transpose(1, 0, 3, 2)
            ).astype(BF16)
        if n_ch:
            vsl = V_cache[:, hs:hs + HPC, :n_ch * 128, :]
            if rem:
                vsl = np.concatenate(
                    [V_cache[:, hs:hs + HPC, :pos, :],
                     np.zeros((B, HPC, n_ch * 128 - pos, HEAD_DIM), np.float32)],
                    axis=2)
            vperm = (vsl.reshape(B, HPC, n_ch, 128, HEAD_DIM)
                     .transpose(1, 0, 3, 2, 4))  # [h, b, p, c, j]
            vaug = np.ones((HPC, B, 128, n_ch, 129), np.float32)
            vaug[..., :HEAD_DIM] = vperm
            m["va"] = vaug.astype(BF16)
        in_maps.append(m)

    if pos not in _PROGRAM_CACHE:
        _PROGRAM_CACHE[pos] = _build_program(pos)
    nc = _PROGRAM_CACHE[pos]

    if TRACE:
        _install_ntff_shim()
    res = run_bass_kernel_spmd(nc, in_maps, list(range(N_CORES)), trace=TRACE)
    LAST_EXEC_NS = res.exec_time_ns

    acc = np.zeros((TOK, HIDDEN), np.float32)
    for r in res.results:
        acc += np.asarray(r["out"], np.float32)
    return acc.reshape(B, S, HIDDEN)
